# revision 1
# baseline (speedup 1.0000x reference)
"""DiffPool (nn_DiffPool_4715874091424) Trainium2 Bass kernel.

Math (reference is jax, B=32, C=CR=32, N=L=160, GDEP=2, ALPHA=0.05):
  A  = rownorm(a + I), A' = rownorm(a.T + I)
  mixprop folding:  embed = E0 x + E1 (M1 x) + E2 (M2 x) + 2 be
                    pool  = P0 x + P1 (M1 x) + P2 (M2 x) + 2 bp
  with M1 = A + A', M2 = A^2 + A'^2 (hop matrices), E*/P* folded 32x32
  channel-mix mats (host precompute).
  s = softmax_v(pool);  x_new[c] = s[c]^T @ embed[c];
  a_new[c] = (s[c] @ a) @ s[c].

Device pipeline per batch element b (8 cores, data-parallel over B, 4 b/core):
  1. x node-major streamed in chunks;  y12 = [M1|M2]^T.T @ x  (node matmuls)
  2. per (v,l)-segment: assemble hcat [96, seg] = [x_chan; y1_chan; y2_chan]
     (y rows via SBUF->SBUF strided DMA = the layout transpose), channel-mix
     matmul (Wcat [96,64]) + bias -> mixout -> DRAM scratch (chan-major)
  3. per c-group: reload pool/embed node-major from scratch, expP=exp(pool);
     x_new-MM with ones-column rhs yields softmax denom D as an extra output
     column; Dinv=1/D; PE-transpose expP -> s^T (Dinv-scaled on evict),
     transpose back -> s; tT = a^T s^T (const stationary); a_new = tT^T s.
"""

import sys

import numpy as np

if "/opt/trn_rl_repo" not in sys.path:
    sys.path.insert(0, "/opt/trn_rl_repo")

import concourse.bass as bass
import concourse.bacc as bacc
import concourse.mybir as mybir
import concourse.tile as tile
from concourse.bass_utils import run_bass_kernel_spmd
from concourse.masks import make_identity

F32 = mybir.dt.float32
F16 = mybir.dt.float16
AF = mybir.ActivationFunctionType

B, C, N, L = 32, 32, 160, 160
NCORES = 8
BPC = B // NCORES  # 4 batch elements per core
ALPHA, BETA = 0.05, 0.95
CL = C * L  # 5120
NSEG = 4
VQ = N // NSEG  # 20 node rows per (v,l) segment
QF = VQ * L  # 3200 free elements per segment
G = 8  # channels per phase-2 group
VT = [(0, 128), (128, 32)]  # partition tiles of the 160 node/cluster dim


def build_nc():
    nc = bacc.Bacc("TRN2", target_bir_lowering=False, debug=False, num_devices=NCORES)
    xs = nc.declare_dram_parameter("xs", [BPC, C, N, L], F16, isOutput=False)
    mt = nc.declare_dram_parameter("mt", [N, 2 * N], F16, isOutput=False)
    wcat = nc.declare_dram_parameter("wcat", [3 * C, 2 * C], F16, isOutput=False)
    b2 = nc.declare_dram_parameter("b2", [2 * C, 1], F32, isOutput=False)
    am = nc.declare_dram_parameter("am", [N, N], F16, isOutput=False)
    xn_out = nc.declare_dram_parameter("xn", [BPC, C, L, L], F32, isOutput=True)
    an_out = nc.declare_dram_parameter("an", [BPC, C, N, L], F32, isOutput=True)
    # scratch: chan-major mix output per b: rows 0:C = embed (fp16),
    # rows C:2C = exp(pool + 2bp) (fp16; exp computed from the fp32 PSUM so
    # no amplified quantization of pool)
    mo = nc.dram_tensor("mo", [BPC, 2 * C, N, L], F16)
    # scratch: chan-major y1/y2 per b: [C, vstack 0:160=y1 160:320=y2, L]
    ys = nc.dram_tensor("ys", [BPC, C, 2 * N, L], F16)

    with tile.TileContext(nc) as tc:
        with (
            tc.tile_pool(name="consts", bufs=1) as pc,
            tc.tile_pool(name="work", bufs=1) as pw,
            tc.tile_pool(name="psum", bufs=1, space="PSUM") as pp,
        ):
            # ---- constants ----
            mt0 = pc.tile([128, 2 * N], F16)
            mt1 = pc.tile([32, 2 * N], F16)
            nc.sync.dma_start(mt0[:], mt[0:128, :])
            nc.sync.dma_start(mt1[:], mt[128:160, :])
            wc = pc.tile([3 * C, 2 * C], F16)
            nc.sync.dma_start(wc[:], wcat[:])
            b2c = pc.tile([2 * C, 1], F32)
            nc.sync.dma_start(b2c[:], b2[:])
            am0 = pc.tile([128, N], F16)
            am1 = pc.tile([32, N], F16)
            nc.sync.dma_start(am0[:], am[0:128, :])
            nc.sync.dma_start(am1[:], am[128:160, :])
            ident = pc.tile([128, 128], F16)
            make_identity(nc, ident[:])

            # contiguous per-Mtile copies of the stationaries (FWL-eligible)
            MTILES = [(0, 128), (128, 128), (256, 64)]
            mtt = []
            for kt, (ksz, mtsrc) in enumerate(((128, mt0), (32, mt1))):
                row = []
                for m0, msz in MTILES:
                    t = pc.tile([ksz, msz], F16, name=f"mtt{kt}_{m0}")
                    nc.vector.tensor_copy(t[:], mtsrc[:, m0 : m0 + msz])
                    row.append(t)
                mtt.append(row)
            amt = []
            for kt, (ksz, asrc) in enumerate(((128, am0), (32, am1))):
                row = []
                for m0, msz in VT:
                    t = pc.tile([ksz, msz], F16, name=f"amt{kt}_{m0}")
                    nc.vector.tensor_copy(t[:], asrc[:, m0 : m0 + msz])
                    row.append(t)
                amt.append(row)

            # PE warm-up: ~10us of back-to-back matmuls to release the HAM
            # clock gate (cold = 1.2 GHz, warm = 2.4 GHz) before real work
            warm = pc.tile([128, 512], F16, name="warm")
            nc.vector.memset(warm[:], 0.125)
            for _ in range(24):
                wps = pp.tile([128, 512], F32, tag="psA", name="psA", bufs=5)
                nc.tensor.matmul(wps[:], warm[:, 0:128], warm[:], start=True, stop=True)

            # software pipeline: emit phase1(b+1) before phase2(b) so the
            # scheduler can fill phase-2 dependency gaps with y12/mix work
            _phase1(nc, pw, pp, xs, mo, ys, 0, mtt, wc, b2c)
            for b in range(BPC):
                if b + 1 < BPC:
                    _phase1(nc, pw, pp, xs, mo, ys, b + 1, mtt, wc, b2c)
                _phase2(nc, pw, pp, mo, xn_out, an_out, b, amt, ident)

    return nc


def _psA(pp, shape, dt=F32):
    return pp.tile(shape, dt, tag="psA", name="psA", bufs=5)


def _psB(pp, shape, dt=F32):
    return pp.tile(shape, dt, tag="psB", name="psB", bufs=3)


def _phase1(nc, pw, pp, xs, mo, ys, b, mtt, wc, b2c):
    """y12 node matmuls + chan-mix -> mo[b] (chan-major)."""
    xnode = xs[b].rearrange("c w l -> w c l")  # [160, 32, 160]
    MTILES = [(0, 128), (128, 128), (256, 64)]

    # y-stack rows: 0:160 = y1 = M1 x, 160:320 = y2 = M2 x
    Y = [
        pw.tile([128, CL], F16, tag="y0", name="y0"),
        pw.tile([128, CL], F16, tag="y1", name="y1"),
        pw.tile([64, CL], F16, tag="y2", name="y2"),
    ]
    # x node-major: one flat [*, 5120] tile per K-tile (single DMA each);
    # rhs chunks of N=512 slice the flat free dim, LDW amortized over 4-chunk
    # groups per stationary
    xc0 = pw.tile([128, CL], F16, tag="xc0", name="xc0", bufs=1)
    xc1 = pw.tile([32, CL], F16, tag="xc1", name="xc1", bufs=1)
    nc.sync.dma_start(xc0[:].rearrange("p (c l) -> p c l", c=C), xnode[0:128])
    nc.sync.dma_start(xc1[:].rearrange("p (c l) -> p c l", c=C), xnode[128:160])
    xcs = [xc0, xc1]
    for mi, (m0, msz) in enumerate(MTILES):
        for sg in range(0, 10, 4):  # chunk groups of <=4 (N=512 each)
            subs = range(sg, min(sg + 4, 10))
            pss = {sub: _psA(pp, [128, 512]) for sub in subs}
            for kt in range(2):
                for sub in subs:
                    nc.tensor.matmul(
                        pss[sub][:msz, :],
                        mtt[kt][mi][:],
                        xcs[kt][:, sub * 512 : (sub + 1) * 512],
                        start=(kt == 0),
                        stop=(kt == 1),
                    )
            for sub in subs:
                nc.any.tensor_copy(
                    Y[mi][:msz, sub * 512 : (sub + 1) * 512], pss[sub][:msz, :]
                )

    # node->chan layout transpose of y12 via DRAM roundtrip (640B runs each way)
    for mi, (m0, msz) in enumerate(MTILES):
        nc.gpsimd.dma_start(
            ys[b][:, m0 : m0 + msz, :].rearrange("c v l -> v c l"),
            Y[mi][:].rearrange("v (c l) -> v c l", c=C),
        )

    # per (v,l)-segment: hcat = [x_chan; y1_chan; y2_chan] [96, QF] -> mix
    for q in range(NSEG):
        v0 = q * VQ
        hq = pw.tile([3 * C, QF], F16, tag="hcat", name="hcat", bufs=2)
        # x rows (chan-major from DRAM)
        nc.sync.dma_start(
            hq[0:C, :].rearrange("c (v l) -> c v l", v=VQ),
            xs[b][:, v0 : v0 + VQ, :],
        )
        # y rows (chan-major from ys scratch)
        for blk, base in ((1, 0), (2, N)):  # hcat block 1 => y1, 2 => y2
            nc.sync.dma_start(
                hq[blk * C : (blk + 1) * C, :].rearrange("c (v l) -> c v l", v=VQ),
                ys[b][:, base + v0 : base + v0 + VQ, :],
            )
        # mix: out[o, pos] = sum_c' wc[c', o] * hq[c', pos], + bias
        moq = pw.tile([2 * C, QF], F16, tag="moq", name="moq", bufs=2)
        for off in range(0, QF, 512):  # chunks of 512 over the flat free dim
            nn = min(512, QF - off)
            ps = _psB(pp, [64, 512])
            nc.tensor.matmul(
                ps[:, :nn], wc[:], hq[:, off : off + nn], start=True, stop=True
            )
            nc.vector.tensor_scalar_add(
                moq[0:C, off : off + nn], ps[0:C, :nn], b2c[0:C, :]
            )
            nc.scalar.activation(
                moq[C : 2 * C, off : off + nn],
                ps[C : 2 * C, :nn],
                AF.Exp,
                bias=b2c[C : 2 * C, :],
            )
        nc.gpsimd.dma_start(
            mo[b][:, v0 : v0 + VQ, :],
            moq[:].rearrange("o (v l) -> o v l", v=VQ),
        )


def _phase2(nc, pw, pp, mo, xn_out, an_out, b, amt, ident):
    """softmax + x_new + a_new per c-group of G."""
    for g in range(C // G):
        c0 = g * G
        # embed (rows 0:32) / expP (rows 32:64) of mo, node-major [v, (c,l)]
        egs, xps = [], []
        for i, (v0, sz) in enumerate(VT):
            eg = pw.tile([sz, G * (L + 1)], F16, tag=f"eg{i}", name=f"eg{i}", bufs=2)
            xp = pw.tile([sz, G * L], F16, tag=f"xp{i}", name=f"xp{i}", bufs=2)
            nc.sync.dma_start(
                eg[:].rearrange("v (c l) -> v c l", c=G)[:, :, 0:L],
                mo[b][c0 : c0 + G, v0 : v0 + sz, :].rearrange("c v l -> v c l"),
            )
            nc.sync.dma_start(
                xp[:].rearrange("v (c l) -> v c l", c=G),
                mo[b][C + c0 : C + c0 + G, v0 : v0 + sz, :].rearrange("c v l -> v c l"),
            )
            nc.vector.memset(
                eg[:].rearrange("v (c l) -> v c l", c=G)[:, :, L : L + 1], 1.0
            )
            egs.append(eg)
            xps.append(xp)
        dvs = [
            pw.tile([sz, G], F32, tag=f"dv{i}", name=f"dv{i}")
            for i, (_, sz) in enumerate(VT)
        ]
        slg = [
            pw.tile([sz, G * N], F16, tag=f"sl{i}", name=f"sl{i}", bufs=2)
            for i, (_, sz) in enumerate(VT)
        ]
        sng = [
            pw.tile([sz, G * L], F16, tag=f"sn{i}", name=f"sn{i}", bufs=2)
            for i, (_, sz) in enumerate(VT)
        ]
        xgs = [
            pw.tile([sz, G * L], F32, tag=f"xg{i}", name=f"xg{i}", bufs=2)
            for i, (_, sz) in enumerate(VT)
        ]

        for ci in range(G):
            # ---- x_new: raw = expP[c]^T @ [e[c] | 1];  D = last col ----
            for mi, (m0, msz) in enumerate(VT):  # l1 tiles
                ps = _psA(pp, [128, L + 1])
                for kt, (k0, ksz) in enumerate(VT):  # v tiles
                    nc.tensor.matmul(
                        ps[:msz, :],
                        xps[kt][:, ci * L + m0 : ci * L + m0 + msz],
                        egs[kt][:, ci * (L + 1) : (ci + 1) * (L + 1)],
                        start=(kt == 0),
                        stop=(kt == 1),
                    )
                nc.vector.reciprocal(dvs[mi][:msz, ci : ci + 1], ps[:msz, L : L + 1])
                nc.scalar.activation(
                    xgs[mi][:msz, ci * L : (ci + 1) * L],
                    ps[:msz, 0:L],
                    AF.Copy,
                    scale=dvs[mi][:msz, ci : ci + 1],
                )
            # ---- T1: s^T[c] = transpose(expP[c]) * Dinv  (l-major) ----
            for kt, (k0, ksz) in enumerate(VT):  # source v tile
                for lt, (l0, lsz) in enumerate(VT):  # source l cols
                    ps = _psA(pp, [128, 128], F16)
                    nc.tensor.transpose(
                        ps[:lsz, :ksz],
                        xps[kt][:, ci * L + l0 : ci * L + l0 + lsz],
                        ident[:ksz, :ksz],
                    )
                    nc.scalar.activation(
                        slg[lt][:lsz, ci * N + k0 : ci * N + k0 + ksz],
                        ps[:lsz, :ksz],
                        AF.Copy,
                        scale=dvs[lt][:lsz, ci : ci + 1],
                    )
            # ---- T2: s[c] = transpose(s^T[c])  (node-major) ----
            for kt, (k0, ksz) in enumerate(VT):  # source l tile
                for vt, (v0, vsz) in enumerate(VT):  # source v cols
                    ps = _psB(pp, [128, 128], F16)
                    nc.tensor.transpose(
                        ps[:vsz, :ksz],
                        slg[kt][:, ci * N + v0 : ci * N + v0 + vsz],
                        ident[:ksz, :ksz],
                    )
                    nc.vector.tensor_copy(
                        sng[vt][:vsz, ci * L + k0 : ci * L + k0 + ksz],
                        ps[:vsz, :ksz],
                    )
        # ---- tT = a^T s^T : tT[j, (c,v)] = sum_k a[k,j] s_l[k, (c,v)] ----
        ttg = [
            pw.tile([sz, G * N], F16, tag=f"tt{i}", name=f"tt{i}", bufs=2)
            for i, (_, sz) in enumerate(VT)
        ]
        NCH = G * N // 320  # chunks of 320
        for mi, (m0, msz) in enumerate(VT):  # j tiles
            for ch in range(NCH):
                ps = _psB(pp, [128, 320])
                for kt in range(2):
                    nc.tensor.matmul(
                        ps[:msz, :],
                        amt[kt][mi][:],
                        slg[kt][:, ch * 320 : (ch + 1) * 320],
                        start=(kt == 0),
                        stop=(kt == 1),
                    )
                nc.any.tensor_copy(
                    ttg[mi][:msz, ch * 320 : (ch + 1) * 320], ps[:msz, :]
                )
        # ---- a_new[c] = tT[c]^T @ s[c] ----
        ang = [
            pw.tile([sz, G * L], F32, tag=f"ag{i}", name=f"ag{i}", bufs=2)
            for i, (_, sz) in enumerate(VT)
        ]
        for ci in range(G):
            for mi, (m0, msz) in enumerate(VT):  # v tiles (output partition)
                ps = _psB(pp, [128, L])
                for kt, (k0, ksz) in enumerate(VT):  # j tiles
                    nc.tensor.matmul(
                        ps[:msz, :],
                        ttg[kt][:, ci * N + m0 : ci * N + m0 + msz],
                        sng[kt][:, ci * L : (ci + 1) * L],
                        start=(kt == 0),
                        stop=(kt == 1),
                    )
                nc.vector.tensor_copy(ang[mi][:msz, ci * L : (ci + 1) * L], ps[:msz, :])
        # ---- outputs ----
        for i, (v0, sz) in enumerate(VT):
            nc.gpsimd.dma_start(
                xn_out[b][c0 : c0 + G, v0 : v0 + sz, :].rearrange("c p q -> p c q"),
                xgs[i][:sz].rearrange("p (c q) -> p c q", c=G),
            )
            nc.gpsimd.dma_start(
                an_out[b][c0 : c0 + G, v0 : v0 + sz, :].rearrange("c p q -> p c q"),
                ang[i][:sz].rearrange("p (c q) -> p c q", c=G),
            )


def _host_prep(x, a, We, be, Wp, bp):
    a = np.asarray(a, np.float64)
    I = np.eye(N, dtype=np.float64)
    A1 = (a + I) / (a + I).sum(1, keepdims=True)
    A2 = (a.T + I) / (a.T + I).sum(1, keepdims=True)
    M1 = A1 + A2
    M2 = A1 @ A1 + A2 @ A2
    MT = np.concatenate([M1.T, M2.T], axis=1).astype(np.float16)  # [N, 2N]

    def fold(W):
        W = np.asarray(W, np.float64)
        W0, W1, W2 = W[:, :C], W[:, C : 2 * C], W[:, 2 * C :]
        F0 = 2.0 * (W0 + ALPHA * W1 + ALPHA * W2)
        F1 = BETA * W1 + ALPHA * BETA * W2
        F2 = BETA * BETA * W2
        return F0, F1, F2

    E0, E1, E2 = fold(We)
    P0, P1, P2 = fold(Wp)
    # lhsT[c', o]: rows = [x-block; y1-block; y2-block], cols = [e outs | pool outs]
    Wcat = np.block([[E0.T, P0.T], [E1.T, P1.T], [E2.T, P2.T]]).astype(np.float16)
    b2 = np.concatenate([2.0 * np.asarray(be), 2.0 * np.asarray(bp)]).astype(
        np.float32
    )[:, None]
    return MT, Wcat, b2, np.asarray(a, np.float16)


def _install_ntff_shim():
    """Provide antenv.axon_hooks (missing in this image) so
    run_bass_kernel_spmd(trace=True) can drive NTFF profiling via the
    axon PJRT .so. No-op if anything is unavailable."""
    import contextlib
    import ctypes
    import types

    try:
        import antenv  # noqa: F401

        try:
            from antenv.axon_hooks import get_axon_ntff_profile_hook  # noqa: F401

            return
        except ImportError:
            pass
        lib = ctypes.CDLL("/opt/axon/libaxon_pjrt.so")
        if not hasattr(lib, "axon_start_nrt_profile"):
            return
        lib.axon_start_nrt_profile.argtypes = [
            ctypes.POINTER(ctypes.c_int64),
            ctypes.c_size_t,
        ]
        lib.axon_start_nrt_profile.restype = ctypes.c_int64
        lib.axon_stop_nrt_profile.argtypes = [ctypes.c_char_p]
        lib.axon_stop_nrt_profile.restype = ctypes.c_int64

        @contextlib.contextmanager
        def _hook(output_dir, device_ids):
            import jax

            jax.devices()
            if device_ids:
                ids = (ctypes.c_int64 * len(device_ids))(*device_ids)
                rc = lib.axon_start_nrt_profile(ids, len(device_ids))
            else:
                rc = lib.axon_start_nrt_profile(None, 0)
            if rc != 0:
                raise RuntimeError(f"axon_start_nrt_profile rc={rc}")
            try:
                yield
            finally:
                n = lib.axon_stop_nrt_profile(str(output_dir).encode())
                print(f"ntff profile: {n} file(s) -> {output_dir}", file=sys.stderr)

        holder = {"h": _hook}
        mod = types.ModuleType("antenv.axon_hooks")
        mod.get_axon_ntff_profile_hook = lambda: holder["h"]
        mod.set_axon_ntff_profile_hook = lambda h: holder.__setitem__("h", h)
        sys.modules["antenv.axon_hooks"] = mod
        antenv.axon_hooks = mod
    except Exception as e:  # pragma: no cover
        print(f"ntff shim unavailable: {e}", file=sys.stderr)


_NC_CACHE = {}


def _get_nc():
    if "nc" not in _NC_CACHE:
        nc = build_nc()
        nc.compile()  # bacc lowering: wait-splitting, register allocation, ...
        _NC_CACHE["nc"] = nc
    return _NC_CACHE["nc"]


def run_spmd(x, a, We, be, Wp, bp, trace=False):
    if trace:
        _install_ntff_shim()
    x16 = np.ascontiguousarray(np.asarray(x, np.float16))
    MT, Wcat, b2, a32 = _host_prep(x, a, We, be, Wp, bp)
    nc = _get_nc()
    in_maps = [
        {
            "xs": x16[i * BPC : (i + 1) * BPC],
            "mt": MT,
            "wcat": Wcat,
            "b2": b2,
            "am": a32,
        }
        for i in range(NCORES)
    ]
    res = run_bass_kernel_spmd(nc, in_maps, list(range(NCORES)), trace=trace)
    xn = np.concatenate([res.results[i]["xn"] for i in range(NCORES)], axis=0)
    an = np.concatenate([res.results[i]["an"] for i in range(NCORES)], axis=0)
    return (xn, an), res


def kernel(x, a, We, be, Wp, bp):
    (xn, an), _ = run_spmd(x, a, We, be, Wp, bp, trace=False)
    return (xn, an)



# revision 2
# speedup vs baseline: 1.0963x; 1.0963x over previous
"""DiffPool (nn_DiffPool_4715874091424) Trainium2 Bass kernel.

Math (reference is jax, B=32, C=CR=32, N=L=160, GDEP=2, ALPHA=0.05):
  A  = rownorm(a + I), A' = rownorm(a.T + I)
  mixprop folding:  embed = E0 x + E1 (M1 x) + E2 (M2 x) + 2 be
                    pool  = P0 x + P1 (M1 x) + P2 (M2 x) + 2 bp
  with M1 = A + A', M2 = A^2 + A'^2 (hop matrices), E*/P* folded 32x32
  channel-mix mats (host precompute).
  s = softmax_v(pool);  x_new[c] = s[c]^T @ embed[c];
  a_new[c] = (s[c] @ a) @ s[c].

Device pipeline per batch element b (8 cores, data-parallel over B, 4 b/core):
  1. x node-major (host-pretransposed, contiguous load); y12 = [M1|M2]^T.T @ x
  2. per v-segment (5 x 32 nodes): hcat [96, seg] = [x_chan; y1_chan; y2_chan]
     (y rows via DRAM roundtrip = the layout transpose), channel-mix matmul
     (Wcat [96,64]) + bias -> embed rows / exp(pool) rows -> DRAM mo (chan-major)
  3. per c-group of 8: reload embed/expP node-major from mo; x_new-MM with
     ones-column rhs yields softmax denom D as an extra column; Dinv=1/D;
     s^T via regular matmul vs identity-rhs (Dinv-scaled on evict; keeps the
     PE HAM activity monitor warm, unlike transpose-mode); tT = a^T s^T
     (const stationary); a_new^T = expP-stationary @ tT (Dinv row-scale on
     evict) -- no second transpose; host swaps a_new^T -> a_new.
  Outputs in fp16 (halves write traffic), host casts to fp32.
"""

import sys

import numpy as np

if "/opt/trn_rl_repo" not in sys.path:
    sys.path.insert(0, "/opt/trn_rl_repo")

import concourse.bass as bass
import concourse.bacc as bacc
import concourse.mybir as mybir
import concourse.tile as tile
from concourse.bass_utils import run_bass_kernel_spmd
from concourse.masks import make_identity

F32 = mybir.dt.float32
F16 = mybir.dt.float16
AF = mybir.ActivationFunctionType

B, C, N, L = 32, 32, 160, 160
NCORES = 8
BPC = B // NCORES  # 4 batch elements per core
ALPHA, BETA = 0.05, 0.95
CL = C * L  # 5120
NSEG = 5
VQ = N // NSEG  # 32 node rows per v-segment
QF = VQ * L  # 5120 free elements per segment
G = 8  # channels per phase-2 group
VT = [(0, 128), (128, 32)]  # partition tiles of the 160 node/cluster dim


def build_nc():
    nc = bacc.Bacc("TRN2", target_bir_lowering=False, debug=False, num_devices=NCORES)
    xs = nc.declare_dram_parameter("xs", [BPC, C, N, L], F16, isOutput=False)
    xnd = nc.declare_dram_parameter("xnd", [BPC, N, C, L], F16, isOutput=False)
    mt = nc.declare_dram_parameter("mt", [N, 2 * N], F16, isOutput=False)
    wcat = nc.declare_dram_parameter("wcat", [3 * C, 2 * C], F16, isOutput=False)
    b2 = nc.declare_dram_parameter("b2", [2 * C, 1], F32, isOutput=False)
    am = nc.declare_dram_parameter("am", [N, N], F16, isOutput=False)
    xn_out = nc.declare_dram_parameter("xn", [BPC, C, L, L], F16, isOutput=True)
    ant_out = nc.declare_dram_parameter("ant", [BPC, C, L, N], F16, isOutput=True)
    # scratch: chan-major mix output per b: rows 0:C = embed (fp16),
    # rows C:2C = exp(pool + 2bp) (fp16; exp computed from the fp32 PSUM so
    # no amplified quantization of pool)
    mo = nc.dram_tensor("mo", [BPC, 2 * C, N, L], F16)
    # scratch: chan-major y1/y2 per b: [C, vstack 0:160=y1 160:320=y2, L]
    ys = nc.dram_tensor("ys", [BPC, C, 2 * N, L], F16)

    with tile.TileContext(nc) as tc:
        with (
            tc.tile_pool(name="consts", bufs=1) as pc,
            tc.tile_pool(name="work", bufs=1) as pw,
            tc.tile_pool(name="psum", bufs=1, space="PSUM") as pp,
        ):
            # ---- prefetch b=0 x node-major before anything else (so the
            # first y12 matmuls land right after warm-up with no PE gap) ----
            xc0 = pw.tile([128, CL], F16, tag="xc0", name="xc0", bufs=2)
            xc1 = pw.tile([32, CL], F16, tag="xc1", name="xc1", bufs=2)
            nc.sync.dma_start(xc0[:], xnd[0, 0:128].rearrange("w c l -> w (c l)"))
            nc.sync.dma_start(xc1[:], xnd[0, 128:160].rearrange("w c l -> w (c l)"))

            # ---- constants ----
            mt0 = pc.tile([128, 2 * N], F16)
            mt1 = pc.tile([32, 2 * N], F16)
            nc.sync.dma_start(mt0[:], mt[0:128, :])
            nc.sync.dma_start(mt1[:], mt[128:160, :])
            wc = pc.tile([3 * C, 2 * C], F16)
            nc.sync.dma_start(wc[:], wcat[:])
            b2c = pc.tile([2 * C, 1], F32)
            nc.sync.dma_start(b2c[:], b2[:])
            am0 = pc.tile([128, N], F16)
            am1 = pc.tile([32, N], F16)
            nc.sync.dma_start(am0[:], am[0:128, :])
            nc.sync.dma_start(am1[:], am[128:160, :])
            # identity-rhs blocks for the s^T matmul-transpose:
            # idk[kt] [ksz, N] holds rows k0:k0+ksz of the NxN identity
            id0 = pc.tile([128, N], F16, name="id0")
            id1 = pc.tile([32, N], F16, name="id1")
            nc.gpsimd.memset(id0[:], 0.0)
            nc.gpsimd.memset(id1[:], 0.0)
            make_identity(nc, id0[:, 0:128], nomemset=True)
            make_identity(nc, id1[:, 128:160], nomemset=True)
            idk = [id0, id1]

            # contiguous per-Mtile copies of the stationaries (FWL-eligible)
            MTILES = [(0, 128), (128, 128), (256, 64)]
            mtt = []
            for kt, (ksz, mtsrc) in enumerate(((128, mt0), (32, mt1))):
                row = []
                for m0, msz in MTILES:
                    t = pc.tile([ksz, msz], F16, name=f"mtt{kt}_{m0}")
                    nc.vector.tensor_copy(t[:], mtsrc[:, m0 : m0 + msz])
                    row.append(t)
                mtt.append(row)
            amt = []
            for kt, (ksz, asrc) in enumerate(((128, am0), (32, am1))):
                row = []
                for m0, msz in VT:
                    t = pc.tile([ksz, msz], F16, name=f"amt{kt}_{m0}")
                    nc.vector.tensor_copy(t[:], asrc[:, m0 : m0 + msz])
                    row.append(t)
                amt.append(row)

            # PE warm-up: back-to-back matmuls to release the HAM clock gate
            # (cold = 1.2 GHz, warm = 2.4 GHz) before real work; also covers
            # the b=0 x prefetch latency.
            warm = pc.tile([128, 512], F16, name="warm")
            nc.vector.memset(warm[:], 0.125)
            for _ in range(28):
                wps = pp.tile([128, 512], F32, tag="psA", name="psA", bufs=5)
                nc.tensor.matmul(wps[:], warm[:, 0:128], warm[:], start=True, stop=True)

            # software pipeline: emit phase1(b+1) before phase2(b) so the
            # scheduler can fill phase-2 dependency gaps with y12/mix work
            _phase1(nc, pw, pp, xs, xnd, mo, ys, 0, mtt, wc, b2c, xc0, xc1)
            for b in range(BPC):
                if b + 1 < BPC:
                    _phase1(nc, pw, pp, xs, xnd, mo, ys, b + 1, mtt, wc, b2c, None, None)
                _phase2(nc, pw, pp, mo, xn_out, ant_out, b, amt, idk)

    return nc


def _psA(pp, shape, dt=F32):
    return pp.tile(shape, dt, tag="psA", name="psA", bufs=5)


def _psB(pp, shape, dt=F32):
    return pp.tile(shape, dt, tag="psB", name="psB", bufs=3)


def _phase1(nc, pw, pp, xs, xnd, mo, ys, b, mtt, wc, b2c, xc0, xc1):
    """y12 node matmuls + chan-mix -> mo[b] (chan-major)."""
    MTILES = [(0, 128), (128, 128), (256, 64)]

    # y-stack rows: 0:160 = y1 = M1 x, 160:320 = y2 = M2 x
    Y = [
        pw.tile([128, CL], F16, tag="y0", name="y0", bufs=2),
        pw.tile([128, CL], F16, tag="y1", name="y1", bufs=2),
        pw.tile([64, CL], F16, tag="y2", name="y2", bufs=2),
    ]
    # x node-major: contiguous load (host pre-transposed xnd)
    if xc0 is None:
        xc0 = pw.tile([128, CL], F16, tag="xc0", name="xc0", bufs=2)
        xc1 = pw.tile([32, CL], F16, tag="xc1", name="xc1", bufs=2)
        nc.sync.dma_start(xc0[:], xnd[b, 0:128].rearrange("w c l -> w (c l)"))
        nc.sync.dma_start(xc1[:], xnd[b, 128:160].rearrange("w c l -> w (c l)"))
    xcs = [xc0, xc1]
    for mi, (m0, msz) in enumerate(MTILES):
        for sg in range(0, 10, 4):  # chunk groups of <=4 (N=512 each)
            subs = range(sg, min(sg + 4, 10))
            pss = {sub: _psA(pp, [128, 512]) for sub in subs}
            for kt in range(2):
                for sub in subs:
                    nc.tensor.matmul(
                        pss[sub][:msz, :],
                        mtt[kt][mi][:],
                        xcs[kt][:, sub * 512 : (sub + 1) * 512],
                        start=(kt == 0),
                        stop=(kt == 1),
                    )
            for sub in subs:
                nc.any.tensor_copy(
                    Y[mi][:msz, sub * 512 : (sub + 1) * 512], pss[sub][:msz, :]
                )

    # node->chan layout transpose of y12 via DRAM roundtrip
    for mi, (m0, msz) in enumerate(MTILES):
        nc.gpsimd.dma_start(
            ys[b][:, m0 : m0 + msz, :].rearrange("c v l -> v c l"),
            Y[mi][:].rearrange("v (c l) -> v c l", c=C),
        )

    # per v-segment: hcat = [x_chan; y1_chan; y2_chan] [96, QF] -> mix
    for q in range(NSEG):
        v0 = q * VQ
        hq = pw.tile([3 * C, QF], F16, tag="hcat", name="hcat", bufs=2)
        # x rows (chan-major from DRAM)
        nc.sync.dma_start(
            hq[0:C, :].rearrange("c (v l) -> c v l", v=VQ),
            xs[b][:, v0 : v0 + VQ, :],
        )
        # y rows (chan-major from ys scratch)
        for blk, base in ((1, 0), (2, N)):  # hcat block 1 => y1, 2 => y2
            nc.sync.dma_start(
                hq[blk * C : (blk + 1) * C, :].rearrange("c (v l) -> c v l", v=VQ),
                ys[b][:, base + v0 : base + v0 + VQ, :],
            )
        # mix: out[o, pos] = sum_c' wc[c', o] * hq[c', pos], + bias
        moq = pw.tile([2 * C, QF], F16, tag="moq", name="moq", bufs=2)
        for off in range(0, QF, 512):  # chunks of 512 over the flat free dim
            ps = _psB(pp, [64, 512])
            nc.tensor.matmul(
                ps[:], wc[:], hq[:, off : off + 512], start=True, stop=True
            )
            nc.vector.tensor_scalar_add(
                moq[0:C, off : off + 512], ps[0:C, :], b2c[0:C, :]
            )
            nc.scalar.activation(
                moq[C : 2 * C, off : off + 512],
                ps[C : 2 * C, :],
                AF.Exp,
                bias=b2c[C : 2 * C, :],
            )
        nc.gpsimd.dma_start(
            mo[b][:, v0 : v0 + VQ, :],
            moq[:].rearrange("o (v l) -> o v l", v=VQ),
        )


def _phase2(nc, pw, pp, mo, xn_out, ant_out, b, amt, idk):
    """softmax + x_new + a_new^T per c-group of G."""
    for g in range(C // G):
        c0 = g * G
        # embed (rows 0:32) / expP (rows 32:64) of mo, node-major [v, (c,l)]
        egs, xps = [], []
        for i, (v0, sz) in enumerate(VT):
            eg = pw.tile([sz, G * (L + 1)], F16, tag=f"eg{i}", name=f"eg{i}", bufs=2)
            xp = pw.tile([sz, G * L], F16, tag=f"xp{i}", name=f"xp{i}", bufs=2)
            nc.sync.dma_start(
                eg[:].rearrange("v (c l) -> v c l", c=G)[:, :, 0:L],
                mo[b][c0 : c0 + G, v0 : v0 + sz, :].rearrange("c v l -> v c l"),
            )
            nc.sync.dma_start(
                xp[:].rearrange("v (c l) -> v c l", c=G),
                mo[b][C + c0 : C + c0 + G, v0 : v0 + sz, :].rearrange("c v l -> v c l"),
            )
            nc.vector.memset(
                eg[:].rearrange("v (c l) -> v c l", c=G)[:, :, L : L + 1], 1.0
            )
            egs.append(eg)
            xps.append(xp)
        dvs = [
            pw.tile([sz, G], F32, tag=f"dv{i}", name=f"dv{i}")
            for i, (_, sz) in enumerate(VT)
        ]
        slg = [
            pw.tile([sz, G * N], F16, tag=f"sl{i}", name=f"sl{i}", bufs=2)
            for i, (_, sz) in enumerate(VT)
        ]
        xgs = [
            pw.tile([sz, G * L], F16, tag=f"xg{i}", name=f"xg{i}", bufs=2)
            for i, (_, sz) in enumerate(VT)
        ]

        for ci in range(G):
            # ---- x_new: raw = expP[c]^T @ [e[c] | 1];  D = last col ----
            for mi, (m0, msz) in enumerate(VT):  # l tiles
                ps = _psA(pp, [128, L + 1])
                for kt, (k0, ksz) in enumerate(VT):  # v tiles
                    nc.tensor.matmul(
                        ps[:msz, :],
                        xps[kt][:, ci * L + m0 : ci * L + m0 + msz],
                        egs[kt][:, ci * (L + 1) : (ci + 1) * (L + 1)],
                        start=(kt == 0),
                        stop=(kt == 1),
                    )
                nc.vector.reciprocal(dvs[mi][:msz, ci : ci + 1], ps[:msz, L : L + 1])
                nc.scalar.activation(
                    xgs[mi][:msz, ci * L : (ci + 1) * L],
                    ps[:msz, 0:L],
                    AF.Copy,
                    scale=dvs[mi][:msz, ci : ci + 1],
                )
            # ---- s^T[c] = transpose(expP[c]) * Dinv via matmul vs identity
            # rhs (regular matmul keeps the HAM activity monitor warm) ----
            for lt, (l0, lsz) in enumerate(VT):  # output l tiles
                ps = _psB(pp, [128, N])
                for kt, (k0, ksz) in enumerate(VT):  # v tiles (contraction)
                    nc.tensor.matmul(
                        ps[:lsz, :],
                        xps[kt][:, ci * L + l0 : ci * L + l0 + lsz],
                        idk[kt][:],
                        start=(kt == 0),
                        stop=(kt == 1),
                    )
                if lt == 0:
                    nc.vector.tensor_scalar_mul(
                        slg[lt][:lsz, ci * N : (ci + 1) * N],
                        ps[:lsz, :],
                        dvs[lt][:lsz, ci : ci + 1],
                    )
                else:
                    nc.scalar.activation(
                        slg[lt][:lsz, ci * N : (ci + 1) * N],
                        ps[:lsz, :],
                        AF.Copy,
                        scale=dvs[lt][:lsz, ci : ci + 1],
                    )
        # ---- tT = a^T s^T : tT[j, (c,v)] = sum_k a[k,j] s_l[k, (c,v)] ----
        ttg = [
            pw.tile([sz, G * N], F16, tag=f"tt{i}", name=f"tt{i}", bufs=2)
            for i, (_, sz) in enumerate(VT)
        ]
        NCH = G * N // 320  # chunks of 320
        for mi, (m0, msz) in enumerate(VT):  # j tiles
            for ch in range(NCH):
                ps = _psB(pp, [128, 320])
                for kt in range(2):
                    nc.tensor.matmul(
                        ps[:msz, :],
                        amt[kt][mi][:],
                        slg[kt][:, ch * 320 : (ch + 1) * 320],
                        start=(kt == 0),
                        stop=(kt == 1),
                    )
                nc.any.tensor_copy(
                    ttg[mi][:msz, ch * 320 : (ch + 1) * 320], ps[:msz, :]
                )
        # ---- a_new^T[c][l, v'] = Dinv[l] * sum_j expP[j, l] tT[j, v'] ----
        ang = [
            pw.tile([sz, G * N], F16, tag=f"ag{i}", name=f"ag{i}", bufs=2)
            for i, (_, sz) in enumerate(VT)
        ]
        for ci in range(G):
            for mi, (m0, msz) in enumerate(VT):  # l tiles (output partition)
                ps = _psA(pp, [128, N])
                for kt, (k0, ksz) in enumerate(VT):  # j tiles (contraction)
                    nc.tensor.matmul(
                        ps[:msz, :],
                        xps[kt][:, ci * L + m0 : ci * L + m0 + msz],
                        ttg[kt][:, ci * N : (ci + 1) * N],
                        start=(kt == 0),
                        stop=(kt == 1),
                    )
                if mi == 0:
                    nc.vector.tensor_scalar_mul(
                        ang[mi][:msz, ci * N : (ci + 1) * N],
                        ps[:msz, :],
                        dvs[mi][:msz, ci : ci + 1],
                    )
                else:
                    nc.scalar.activation(
                        ang[mi][:msz, ci * N : (ci + 1) * N],
                        ps[:msz, :],
                        AF.Copy,
                        scale=dvs[mi][:msz, ci : ci + 1],
                    )
        # ---- outputs (fp16; host casts / swaps) ----
        for i, (v0, sz) in enumerate(VT):
            nc.gpsimd.dma_start(
                xn_out[b][c0 : c0 + G, v0 : v0 + sz, :].rearrange("c p q -> p c q"),
                xgs[i][:sz].rearrange("p (c q) -> p c q", c=G),
            )
            nc.gpsimd.dma_start(
                ant_out[b][c0 : c0 + G, v0 : v0 + sz, :].rearrange("c p q -> p c q"),
                ang[i][:sz].rearrange("p (c q) -> p c q", c=G),
            )


def _host_prep(x, a, We, be, Wp, bp):
    a = np.asarray(a, np.float64)
    I = np.eye(N, dtype=np.float64)
    A1 = (a + I) / (a + I).sum(1, keepdims=True)
    A2 = (a.T + I) / (a.T + I).sum(1, keepdims=True)
    M1 = A1 + A2
    M2 = A1 @ A1 + A2 @ A2
    MT = np.concatenate([M1.T, M2.T], axis=1).astype(np.float16)  # [N, 2N]

    def fold(W):
        W = np.asarray(W, np.float64)
        W0, W1, W2 = W[:, :C], W[:, C : 2 * C], W[:, 2 * C :]
        F0 = 2.0 * (W0 + ALPHA * W1 + ALPHA * W2)
        F1 = BETA * W1 + ALPHA * BETA * W2
        F2 = BETA * BETA * W2
        return F0, F1, F2

    E0, E1, E2 = fold(We)
    P0, P1, P2 = fold(Wp)
    # lhsT[c', o]: rows = [x-block; y1-block; y2-block], cols = [e outs | pool outs]
    Wcat = np.block([[E0.T, P0.T], [E1.T, P1.T], [E2.T, P2.T]]).astype(np.float16)
    b2 = np.concatenate([2.0 * np.asarray(be), 2.0 * np.asarray(bp)]).astype(
        np.float32
    )[:, None]
    return MT, Wcat, b2, np.asarray(a, np.float16)


def _install_ntff_shim():
    """Provide antenv.axon_hooks (missing in this image) so
    run_bass_kernel_spmd(trace=True) can drive NTFF profiling via the
    axon PJRT .so. No-op if anything is unavailable."""
    import contextlib
    import ctypes
    import types

    try:
        import antenv  # noqa: F401

        try:
            from antenv.axon_hooks import get_axon_ntff_profile_hook  # noqa: F401

            return
        except ImportError:
            pass
        lib = ctypes.CDLL("/opt/axon/libaxon_pjrt.so")
        if not hasattr(lib, "axon_start_nrt_profile"):
            return
        lib.axon_start_nrt_profile.argtypes = [
            ctypes.POINTER(ctypes.c_int64),
            ctypes.c_size_t,
        ]
        lib.axon_start_nrt_profile.restype = ctypes.c_int64
        lib.axon_stop_nrt_profile.argtypes = [ctypes.c_char_p]
        lib.axon_stop_nrt_profile.restype = ctypes.c_int64

        @contextlib.contextmanager
        def _hook(output_dir, device_ids):
            import jax

            jax.devices()
            if device_ids:
                ids = (ctypes.c_int64 * len(device_ids))(*device_ids)
                rc = lib.axon_start_nrt_profile(ids, len(device_ids))
            else:
                rc = lib.axon_start_nrt_profile(None, 0)
            if rc != 0:
                raise RuntimeError(f"axon_start_nrt_profile rc={rc}")
            try:
                yield
            finally:
                n = lib.axon_stop_nrt_profile(str(output_dir).encode())
                print(f"ntff profile: {n} file(s) -> {output_dir}", file=sys.stderr)

        holder = {"h": _hook}
        mod = types.ModuleType("antenv.axon_hooks")
        mod.get_axon_ntff_profile_hook = lambda: holder["h"]
        mod.set_axon_ntff_profile_hook = lambda h: holder.__setitem__("h", h)
        sys.modules["antenv.axon_hooks"] = mod
        antenv.axon_hooks = mod
    except Exception as e:  # pragma: no cover
        print(f"ntff shim unavailable: {e}", file=sys.stderr)


_NC_CACHE = {}


def _get_nc():
    if "nc" not in _NC_CACHE:
        nc = build_nc()
        nc.compile()  # bacc lowering: wait-splitting, register allocation, ...
        _NC_CACHE["nc"] = nc
    return _NC_CACHE["nc"]


def run_spmd(x, a, We, be, Wp, bp, trace=False):
    if trace:
        _install_ntff_shim()
    x16 = np.ascontiguousarray(np.asarray(x, np.float16))
    xnd = np.ascontiguousarray(x16.transpose(0, 2, 1, 3))  # [B, N, C, L]
    MT, Wcat, b2, a16 = _host_prep(x, a, We, be, Wp, bp)
    nc = _get_nc()
    in_maps = [
        {
            "xs": x16[i * BPC : (i + 1) * BPC],
            "xnd": xnd[i * BPC : (i + 1) * BPC],
            "mt": MT,
            "wcat": Wcat,
            "b2": b2,
            "am": a16,
        }
        for i in range(NCORES)
    ]
    res = run_bass_kernel_spmd(nc, in_maps, list(range(NCORES)), trace=trace)
    xn = np.concatenate([res.results[i]["xn"] for i in range(NCORES)], axis=0)
    ant = np.concatenate([res.results[i]["ant"] for i in range(NCORES)], axis=0)
    xn = xn.astype(np.float32)
    an = ant.swapaxes(-1, -2).astype(np.float32)
    return (xn, an), res


def kernel(x, a, We, be, Wp, bp):
    (xn, an), _ = run_spmd(x, a, We, be, Wp, bp, trace=False)
    return (xn, an)


# revision 14
# speedup vs baseline: 1.1116x; 1.0139x over previous
"""DiffPool (nn_DiffPool_4715874091424) Trainium2 Bass kernel.

Math (reference is jax, B=32, C=CR=32, N=L=160, GDEP=2, ALPHA=0.05):
  A  = rownorm(a + I), A' = rownorm(a.T + I)
  mixprop folding:  embed = E0 x + E1 (M1 x) + E2 (M2 x) + 2 be
                    pool  = P0 x + P1 (M1 x) + P2 (M2 x) + 2 bp
  with M1 = A + A', M2 = A^2 + A'^2 (hop matrices), E*/P* folded 32x32
  channel-mix mats (host precompute).
  s = softmax_v(pool);  x_new[c] = s[c]^T @ embed[c];
  a_new[c] = (s[c] @ a) @ s[c].

Device pipeline per batch element b (8 cores, data-parallel over B, 4 b/core):
  1. x node-major (host-pretransposed, contiguous load); y12 = [M1|M2]^T.T @ x
  2. per v-segment (5 x 32 nodes): hcat [96, seg] = [x_chan; y1_chan; y2_chan]
     (y rows via DRAM roundtrip = the layout transpose), channel-mix matmul
     (Wcat [96,64]) + bias -> embed rows / exp(pool - ln64) rows -> DRAM mo
  3. per c-group of 8: reload embed/expP node-major from mo; x_new raw with
     ones-column rhs (softmax denom D rides as col 161); one strided recip
     per l-tile -> Dinv; s^T raw via matmul vs identity rhs, scaled by a
     single stride-0-broadcast tensor_tensor; tT = a^T s^T (const stationary);
     a_new^T raw = expP-stationary @ tT.  Evictions are merged 3-channels-
     per-PSUM-bank plain copies, alternating ACT/DVE.
  Outputs xn / a_new^T / Dinv in fp16/fp32; HOST applies the Dinv row scale
  and the final a_new transpose (softmax shift-invariance makes the -ln64
  shift cancel).  Phase-1 work for b+1 is rationed across phase-2 groups of
  b to keep the PE HAM activity monitor warm; dummy-matmul bursts at batch
  boundaries re-warm the clock gate if a stall window slipped through.
"""

import sys

import numpy as np

if "/opt/trn_rl_repo" not in sys.path:
    sys.path.insert(0, "/opt/trn_rl_repo")

import concourse.bass as bass
import concourse.bacc as bacc
import concourse.mybir as mybir
import concourse.tile as tile
from concourse.bass import AP
from concourse.bass_utils import run_bass_kernel_spmd
from concourse.masks import make_identity

F32 = mybir.dt.float32
F16 = mybir.dt.float16
AF = mybir.ActivationFunctionType
MUL = mybir.AluOpType.mult

B, C, N, L = 32, 32, 160, 160
NCORES = 8
BPC = B // NCORES  # 4 batch elements per core
ALPHA, BETA = 0.05, 0.95
LN_SHIFT = float(np.log(1024.0))  # softmax shift: keeps raw fp16 outputs in range
CL = C * L  # 5120
NSEG = 5
VQ = N // NSEG  # 32 node rows per v-segment
QF = VQ * L  # 5120 free elements per segment
G = 8  # channels per phase-2 group
NGRP = C // G
VT = [(0, 128), (128, 32)]  # partition tiles of the 160 node/cluster dim
TRIPLES = [(0, 3), (3, 3), (6, 2)]  # 3-channel psum-bank packing of G=8


class _Evict:
    """Alternate PSUM evictions between DVE and ACT."""

    def __init__(self, nc):
        self.nc = nc
        self.i = 0

    def copy(self, out, in_):
        if self.i % 2 == 0:
            self.nc.vector.tensor_copy(out, in_)
        else:
            self.nc.scalar.activation(out, in_, AF.Copy)
        self.i += 1


def build_nc():
    nc = bacc.Bacc("TRN2", target_bir_lowering=False, debug=False, num_devices=NCORES)
    xs = nc.declare_dram_parameter("xs", [BPC, C, N, L], F16, isOutput=False)
    xnd = nc.declare_dram_parameter("xnd", [BPC, N, C, L], F16, isOutput=False)
    mt = nc.declare_dram_parameter("mt", [N, 2 * N], F16, isOutput=False)
    wcat = nc.declare_dram_parameter("wcat", [3 * C, 2 * C], F16, isOutput=False)
    b2 = nc.declare_dram_parameter("b2", [2 * C, 1], F32, isOutput=False)
    am = nc.declare_dram_parameter("am", [N, N], F16, isOutput=False)
    xn_out = nc.declare_dram_parameter("xn", [BPC, C, L, L], F16, isOutput=True)
    ant_out = nc.declare_dram_parameter("ant", [BPC, C, L, N], F16, isOutput=True)
    dv_out = nc.declare_dram_parameter("dv", [BPC, L, C], F32, isOutput=True)
    mo = nc.dram_tensor("mo", [BPC, 2 * C, N, L], F16)
    ys = nc.dram_tensor("ys", [BPC, C, 2 * N, L], F16)

    ev = _Evict(nc)

    with tile.TileContext(nc) as tc:
        with (
            tc.tile_pool(name="consts", bufs=1) as pc,
            tc.tile_pool(name="work", bufs=1) as pw,
            tc.tile_pool(name="psum", bufs=1, space="PSUM") as pp,
        ):
            # ---- prefetch b=0 x node-major before anything else ----
            xc = _xc_load(nc, pw, xnd, 0)

            # ---- constants ----
            mt0 = pc.tile([128, 2 * N], F16)
            mt1 = pc.tile([32, 2 * N], F16)
            nc.sync.dma_start(mt0[:], mt[0:128, :])
            nc.sync.dma_start(mt1[:], mt[128:160, :])
            wc = pc.tile([3 * C, 2 * C], F16)
            nc.sync.dma_start(wc[:], wcat[:])
            b2c = pc.tile([2 * C, 1], F32)
            nc.sync.dma_start(b2c[:], b2[:])
            am0 = pc.tile([128, N], F16)
            am1 = pc.tile([32, N], F16)
            nc.sync.dma_start(am0[:], am[0:128, :])
            nc.sync.dma_start(am1[:], am[128:160, :])
            # identity-rhs blocks with a trailing ones column: the s^T
            # matmul-transpose then also emits the softmax denominator D as
            # output column 160 (accumulated over both K tiles).
            id0 = pc.tile([128, N + 2], F16, name="id0")
            id1 = pc.tile([32, N + 2], F16, name="id1")
            nc.gpsimd.memset(id0[:], 0.0)
            nc.gpsimd.memset(id1[:], 0.0)
            make_identity(nc, id0[:, 0:128], nomemset=True)
            make_identity(nc, id1[:, 128:160], nomemset=True)
            nc.vector.memset(id0[:, N : N + 1], 1.0)
            nc.vector.memset(id1[:, N : N + 1], 1.0)
            idk = [id0, id1]

            MTILES = [(0, 128), (128, 128), (256, 64)]
            mtt = []
            for kt, (ksz, mtsrc) in enumerate(((128, mt0), (32, mt1))):
                row = []
                for m0, msz in MTILES:
                    t = pc.tile([ksz, msz], F16, name=f"mtt{kt}_{m0}")
                    nc.vector.tensor_copy(t[:], mtsrc[:, m0 : m0 + msz])
                    row.append(t)
                mtt.append(row)
            amt = []
            for kt, (ksz, asrc) in enumerate(((128, am0), (32, am1))):
                row = []
                for m0, msz in VT:
                    t = pc.tile([ksz, msz], F16, name=f"amt{kt}_{m0}")
                    nc.vector.tensor_copy(t[:], asrc[:, m0 : m0 + msz])
                    row.append(t)
                amt.append(row)

            warm = pc.tile([128, 512], F16, name="warm")
            nc.vector.memset(warm[:], 0.125)
            _warm_burst(nc, pp, warm, 28)

            st = {"mtt": mtt, "wc": wc, "b2c": b2c, "amt": amt, "idk": idk,
                  "warm": warm, "xc": {0: xc}}

            # ---- prime: full phase1(b=0) ----
            y = _y12(nc, pw, pp, st, 0)
            _ys_dma(nc, ys, y, 0)
            for q in range(NSEG):
                _mix_seg(nc, pw, pp, st, xs, ys, mo, 0, q)
            ld = _ph2_loads(nc, pw, mo, 0, 0)

            # ---- steady state: phase2(b) groups with phase1(b+1) rationed ----
            for b in range(BPC):
                dvall = [
                    pw.tile([sz, C], F32, tag=f"dva{i}", name=f"dva{i}", bufs=2)
                    for i, (_, sz) in enumerate(VT)
                ]
                for g in range(NGRP):
                    if b + 1 < BPC:
                        if g == 0:
                            st["xc"][b + 1] = _xc_load(nc, pw, xnd, b + 1)
                            yb = _y12_part(nc, pw, pp, st, b + 1, 0)
                        elif g == 1:
                            _y12_part(nc, pw, pp, st, b + 1, 1, yb)
                        elif g == 2:
                            _y12_part(nc, pw, pp, st, b + 1, 2, yb)
                            _ys_dma(nc, ys, yb, b + 1)
                        elif g == 3:
                            for q in range(NSEG):
                                _mix_seg(nc, pw, pp, st, xs, ys, mo, b + 1, q)
                    if g == 0:
                        _warm_burst(nc, pp, warm, 8)  # re-warm insurance
                    nxt = (b, g + 1) if g + 1 < NGRP else (b + 1, 0)
                    ld_next = (
                        _ph2_loads(nc, pw, mo, nxt[0], nxt[1])
                        if nxt[0] < BPC
                        else None
                    )
                    _ph2_group(nc, pw, pp, st, ld, dvall, xn_out, ant_out, b, g, ev)
                    ld = ld_next
                for i, (v0, sz) in enumerate(VT):
                    nc.scalar.dma_start(dv_out[b][v0 : v0 + sz, :], dvall[i][:])

    return nc


def _psA(pp, shape, dt=F32):
    return pp.tile(shape, dt, tag="psA", name="psA", bufs=4)


def _psB(pp, shape, dt=F32):
    return pp.tile(shape, dt, tag="psB", name="psB", bufs=4)


def _warm_burst(nc, pp, warm, n):
    for _ in range(n):
        wps = _psA(pp, [128, 512])
        nc.tensor.matmul(wps[:], warm[:, 0:128], warm[:], start=True, stop=True)


def _xc_load(nc, pw, xnd, b):
    xc0 = pw.tile([128, CL], F16, tag="xc0", name="xc0", bufs=2)
    xc1 = pw.tile([32, CL], F16, tag="xc1", name="xc1", bufs=2)
    nc.sync.dma_start(xc0[:], xnd[b, 0:128].rearrange("w c l -> w (c l)"))
    nc.sync.dma_start(xc1[:], xnd[b, 128:160].rearrange("w c l -> w (c l)"))
    return [xc0, xc1]


def _y12_alloc(pw):
    return [
        pw.tile([128, CL], F16, tag="y0", name="y0", bufs=1),
        pw.tile([128, CL], F16, tag="y1", name="y1", bufs=1),
        pw.tile([64, CL], F16, tag="y2", name="y2", bufs=1),
    ]


def _y12_part(nc, pw, pp, st, b, mi, Y=None):
    """One M-tile of the y12 node matmul."""
    MTILES = [(0, 128), (128, 128), (256, 64)]
    if Y is None:
        Y = _y12_alloc(pw)
    xcs = st["xc"][b]
    mtt = st["mtt"]
    m0, msz = MTILES[mi]
    for sg in range(0, 10, 4):  # chunk groups of <=4 (512 cols each)
        subs = range(sg, min(sg + 4, 10))
        pss = {sub: _psA(pp, [128, 512]) for sub in subs}
        for kt in range(2):
            for sub in subs:
                nc.tensor.matmul(
                    pss[sub][:msz, :],
                    mtt[kt][mi][:],
                    xcs[kt][:, sub * 512 : (sub + 1) * 512],
                    start=(kt == 0),
                    stop=(kt == 1),
                )
        for j, sub in enumerate(subs):
            if sub % 2 == 0:
                nc.vector.tensor_copy(
                    Y[mi][:msz, sub * 512 : (sub + 1) * 512], pss[sub][:msz, :]
                )
            else:
                nc.scalar.activation(
                    Y[mi][:msz, sub * 512 : (sub + 1) * 512],
                    pss[sub][:msz, :],
                    AF.Copy,
                )
    return Y


def _y12(nc, pw, pp, st, b):
    Y = _y12_alloc(pw)
    for mi in range(3):
        _y12_part(nc, pw, pp, st, b, mi, Y)
    return Y


def _ys_dma(nc, ys, Y, b):
    MTILES = [(0, 128), (128, 128), (256, 64)]
    for mi, (m0, msz) in enumerate(MTILES):
        nc.gpsimd.dma_start(
            ys[b][:, m0 : m0 + msz, :].rearrange("c v l -> v c l"),
            Y[mi][:].rearrange("v (c l) -> v c l", c=C),
        )


def _mix_seg(nc, pw, pp, st, xs, ys, mo, b, q):
    """hcat assembly + channel mix for one v-segment."""
    wc, b2c = st["wc"], st["b2c"]
    v0 = q * VQ
    hq = pw.tile([3 * C, QF], F16, tag="hcat", name="hcat", bufs=2)
    nc.sync.dma_start(
        hq[0:C, :].rearrange("c (v l) -> c v l", v=VQ),
        xs[b][:, v0 : v0 + VQ, :],
    )
    for blk, base in ((1, 0), (2, N)):
        nc.sync.dma_start(
            hq[blk * C : (blk + 1) * C, :].rearrange("c (v l) -> c v l", v=VQ),
            ys[b][:, base + v0 : base + v0 + VQ, :],
        )
    moq = pw.tile([2 * C, QF], F16, tag="moq", name="moq", bufs=2)
    for off in range(0, QF, 512):
        ps = _psB(pp, [64, 512])
        nc.tensor.matmul(ps[:], wc[:], hq[:, off : off + 512], start=True, stop=True)
        nc.vector.tensor_scalar_add(moq[0:C, off : off + 512], ps[0:C, :], b2c[0:C, :])
        nc.scalar.activation(
            moq[C : 2 * C, off : off + 512],
            ps[C : 2 * C, :],
            AF.Exp,
            bias=b2c[C : 2 * C, :],
        )
    nc.gpsimd.dma_start(
        mo[b][:, v0 : v0 + VQ, :],
        moq[:].rearrange("o (v l) -> o v l", v=VQ),
    )


def _ph2_loads(nc, pw, mo, b, g):
    """Load embed and expP node-major tiles for one c-group."""
    c0 = g * G
    egs, xps = [], []
    for i, (v0, sz) in enumerate(VT):
        eg = pw.tile([sz, G * L], F16, tag=f"eg{i}", name=f"eg{i}", bufs=2)
        xp = pw.tile([sz, G * L], F16, tag=f"xp{i}", name=f"xp{i}", bufs=2)
        nc.sync.dma_start(
            eg[:].rearrange("v (c l) -> v c l", c=G),
            mo[b][c0 : c0 + G, v0 : v0 + sz, :].rearrange("c v l -> v c l"),
        )
        nc.sync.dma_start(
            xp[:].rearrange("v (c l) -> v c l", c=G),
            mo[b][C + c0 : C + c0 + G, v0 : v0 + sz, :].rearrange("c v l -> v c l"),
        )
        egs.append(eg)
        xps.append(xp)
    return egs, xps


def _bcast(ap2d, nfree):
    """Append a stride-0 free dim of size nfree to a [p, c] AP."""
    return AP(ap2d.tensor, ap2d.offset, list(ap2d.ap) + [[0, nfree]])


def _ph2_group(nc, pw, pp, st, ld, dvall, xn_out, ant_out, b, g, ev):
    amt, idk = st["amt"], st["idk"]
    egs, xps = ld
    c0 = g * G

    SW = N + 2  # even channel stride for the s^T-raw + D layout (psum 8B lines)
    xraw = [
        pw.tile([sz, G * L], F16, tag=f"xr{i}", name=f"xr{i}", bufs=2)
        for i, (_, sz) in enumerate(VT)
    ]
    slgr = [
        pw.tile([sz, G * SW], F16, tag=f"sr{i}", name=f"sr{i}", bufs=2)
        for i, (_, sz) in enumerate(VT)
    ]
    slg = [
        pw.tile([sz, G * N], F16, tag=f"sl{i}", name=f"sl{i}", bufs=2)
        for i, (_, sz) in enumerate(VT)
    ]
    ttg = [
        pw.tile([sz, G * N], F16, tag=f"tt{i}", name=f"tt{i}", bufs=2)
        for i, (_, sz) in enumerate(VT)
    ]
    angr = [
        pw.tile([sz, G * N], F16, tag=f"ag{i}", name=f"ag{i}", bufs=2)
        for i, (_, sz) in enumerate(VT)
    ]

    # ---- s^T raw (+ D in col 160) via matmul vs identity|ones rhs ----
    for lt, (l0, lsz) in enumerate(VT):  # output l tiles
        for t0, nch in TRIPLES:
            ps = _psB(pp, [128, 512])
            for j in range(nch):
                ci = t0 + j
                for kt, (k0, ksz) in enumerate(VT):  # v tiles (contraction)
                    nc.tensor.matmul(
                        ps[:lsz, j * SW : (j + 1) * SW],
                        xps[kt][:, ci * L + l0 : ci * L + l0 + lsz],
                        idk[kt][:],
                        start=(kt == 0),
                        stop=(kt == 1),
                    )
            ev.copy(
                slgr[lt][:lsz, t0 * SW : (t0 + nch) * SW], ps[:lsz, 0 : nch * SW]
            )
        # one strided reciprocal per l-tile: Dinv for all 8 channels
        nc.vector.reciprocal(
            dvall[lt][:lsz, c0 : c0 + G],
            slgr[lt][:lsz].rearrange("p (c q) -> p c q", c=G)[:, :, N],
        )
        nc.vector.tensor_tensor(
            slg[lt][:lsz].rearrange("p (c v) -> p c v", c=G),
            slgr[lt][:lsz].rearrange("p (c q) -> p c q", c=G)[:, :, 0:N],
            _bcast(dvall[lt][:lsz, c0 : c0 + G], N),
            MUL,
        )

    # ---- tT = a^T s^T ----
    for mi, (m0, msz) in enumerate(VT):  # j tiles
        for cf0, csz in ((0, 512), (512, 512), (1024, 256)):
            ps = _psB(pp, [128, 512])
            for kt in range(2):
                nc.tensor.matmul(
                    ps[:msz, :csz],
                    amt[kt][mi][:],
                    slg[kt][:, cf0 : cf0 + csz],
                    start=(kt == 0),
                    stop=(kt == 1),
                )
            ev.copy(ttg[mi][:msz, cf0 : cf0 + csz], ps[:msz, :csz])

    # ---- x_new raw + a_new^T raw, interleaved: adjacent matmuls share the
    # same expP stationary slice, doubling moving data per weight load ----
    for mi, (m0, msz) in enumerate(VT):  # l tiles (output partition)
        for t0, nch in TRIPLES:
            psx = _psA(pp, [128, 512])
            psa = _psA(pp, [128, 512])
            for j in range(nch):
                ci = t0 + j
                for kt, (k0, ksz) in enumerate(VT):  # v/j tiles (contraction)
                    stat = xps[kt][:, ci * L + m0 : ci * L + m0 + msz]
                    nc.tensor.matmul(
                        psx[:msz, j * L : (j + 1) * L],
                        stat,
                        egs[kt][:, ci * L : (ci + 1) * L],
                        start=(kt == 0),
                        stop=(kt == 1),
                    )
                    nc.tensor.matmul(
                        psa[:msz, j * N : (j + 1) * N],
                        stat,
                        ttg[kt][:, ci * N : (ci + 1) * N],
                        start=(kt == 0),
                        stop=(kt == 1),
                    )
            ev.copy(xraw[mi][:msz, t0 * L : (t0 + nch) * L], psx[:msz, 0 : nch * L])
            ev.copy(angr[mi][:msz, t0 * N : (t0 + nch) * N], psa[:msz, 0 : nch * N])

    # ---- outputs (raw fp16; host applies Dinv scale / transpose) ----
    for i, (v0, sz) in enumerate(VT):
        nc.scalar.dma_start(
            xn_out[b][c0 : c0 + G, v0 : v0 + sz, :].rearrange("c p q -> p c q"),
            xraw[i][:sz].rearrange("p (c q) -> p c q", c=G),
        )
        nc.scalar.dma_start(
            ant_out[b][c0 : c0 + G, v0 : v0 + sz, :].rearrange("c p q -> p c q"),
            angr[i][:sz].rearrange("p (c q) -> p c q", c=G),
        )


def _host_prep(x, a, We, be, Wp, bp):
    a = np.asarray(a, np.float64)
    I = np.eye(N, dtype=np.float64)
    A1 = (a + I) / (a + I).sum(1, keepdims=True)
    A2 = (a.T + I) / (a.T + I).sum(1, keepdims=True)
    M1 = A1 + A2
    M2 = A1 @ A1 + A2 @ A2
    MT = np.concatenate([M1.T, M2.T], axis=1).astype(np.float16)  # [N, 2N]

    def fold(W):
        W = np.asarray(W, np.float64)
        W0, W1, W2 = W[:, :C], W[:, C : 2 * C], W[:, 2 * C :]
        F0 = 2.0 * (W0 + ALPHA * W1 + ALPHA * W2)
        F1 = BETA * W1 + ALPHA * BETA * W2
        F2 = BETA * BETA * W2
        return F0, F1, F2

    E0, E1, E2 = fold(We)
    P0, P1, P2 = fold(Wp)
    Wcat = np.block([[E0.T, P0.T], [E1.T, P1.T], [E2.T, P2.T]]).astype(np.float16)
    b2 = np.concatenate(
        [2.0 * np.asarray(be), 2.0 * np.asarray(bp) - LN_SHIFT]
    ).astype(np.float32)[:, None]
    return MT, Wcat, b2, np.asarray(a, np.float16)


def _postprocess(xn_raw, ant_raw, dv):
    # dv: [*, L, C] Dinv values; raw outputs are scaled by Dinv along their
    # l (row) dim, then a_new^T is transposed back.
    dinv = dv.transpose(0, 2, 1)[:, :, :, None]  # [*, C, L, 1]
    xn = xn_raw.astype(np.float32) * dinv
    an = (ant_raw.astype(np.float32) * dinv).swapaxes(-1, -2)
    return np.ascontiguousarray(xn), np.ascontiguousarray(an)


def _install_ntff_shim():
    """Provide antenv.axon_hooks (missing in this image) so
    run_bass_kernel_spmd(trace=True) can drive NTFF profiling via the
    axon PJRT .so. No-op if anything is unavailable."""
    import contextlib
    import ctypes
    import types

    try:
        import antenv  # noqa: F401

        try:
            from antenv.axon_hooks import get_axon_ntff_profile_hook  # noqa: F401

            return
        except ImportError:
            pass
        lib = ctypes.CDLL("/opt/axon/libaxon_pjrt.so")
        if not hasattr(lib, "axon_start_nrt_profile"):
            return
        lib.axon_start_nrt_profile.argtypes = [
            ctypes.POINTER(ctypes.c_int64),
            ctypes.c_size_t,
        ]
        lib.axon_start_nrt_profile.restype = ctypes.c_int64
        lib.axon_stop_nrt_profile.argtypes = [ctypes.c_char_p]
        lib.axon_stop_nrt_profile.restype = ctypes.c_int64

        @contextlib.contextmanager
        def _hook(output_dir, device_ids):
            import jax

            jax.devices()
            if device_ids:
                ids = (ctypes.c_int64 * len(device_ids))(*device_ids)
                rc = lib.axon_start_nrt_profile(ids, len(device_ids))
            else:
                rc = lib.axon_start_nrt_profile(None, 0)
            if rc != 0:
                raise RuntimeError(f"axon_start_nrt_profile rc={rc}")
            try:
                yield
            finally:
                n = lib.axon_stop_nrt_profile(str(output_dir).encode())
                print(f"ntff profile: {n} file(s) -> {output_dir}", file=sys.stderr)

        holder = {"h": _hook}
        mod = types.ModuleType("antenv.axon_hooks")
        mod.get_axon_ntff_profile_hook = lambda: holder["h"]
        mod.set_axon_ntff_profile_hook = lambda h: holder.__setitem__("h", h)
        sys.modules["antenv.axon_hooks"] = mod
        antenv.axon_hooks = mod
    except Exception as e:  # pragma: no cover
        print(f"ntff shim unavailable: {e}", file=sys.stderr)


_NC_CACHE = {}


def _get_nc():
    if "nc" not in _NC_CACHE:
        nc = build_nc()
        nc.compile()
        _NC_CACHE["nc"] = nc
    return _NC_CACHE["nc"]


def run_spmd(x, a, We, be, Wp, bp, trace=False):
    if trace:
        _install_ntff_shim()
    x16 = np.ascontiguousarray(np.asarray(x, np.float16))
    xnd = np.ascontiguousarray(x16.transpose(0, 2, 1, 3))  # [B, N, C, L]
    MT, Wcat, b2, a16 = _host_prep(x, a, We, be, Wp, bp)
    nc = _get_nc()
    in_maps = [
        {
            "xs": x16[i * BPC : (i + 1) * BPC],
            "xnd": xnd[i * BPC : (i + 1) * BPC],
            "mt": MT,
            "wcat": Wcat,
            "b2": b2,
            "am": a16,
        }
        for i in range(NCORES)
    ]
    res = run_bass_kernel_spmd(nc, in_maps, list(range(NCORES)), trace=trace)
    xn_raw = np.concatenate([res.results[i]["xn"] for i in range(NCORES)], axis=0)
    ant_raw = np.concatenate([res.results[i]["ant"] for i in range(NCORES)], axis=0)
    dv = np.concatenate([res.results[i]["dv"] for i in range(NCORES)], axis=0)
    xn, an = _postprocess(xn_raw, ant_raw, dv)
    return (xn, an), res


def kernel(x, a, We, be, Wp, bp):
    (xn, an), _ = run_spmd(x, a, We, be, Wp, bp, trace=False)
    return (xn, an)
